# revision 16
# baseline (speedup 1.0000x reference)
"""Trainium2 Bass kernel for batched Jacobi iteration (5-point Laplacian).

Reference computation:
    x <- invD * (b - M x)   repeated `maxiter` times,
where M is the off-diagonal part of the 5-point Laplacian on a 512x512
grid, given in COO form.  For the actual inputs M is exactly the
4-neighbor stencil with value -1 and invD == 0.25, so the update is

    x_new[r, c] = 0.25 * (b[r, c] + x[r-1,c] + x[r+1,c] + x[r,c-1] + x[r,c+1])

(missing neighbors at grid edges contribute 0).

Strategy (8 NeuronCores, data parallel over batch B=16 -> 2 per core):
  - whole working set lives in SBUF for all iterations; state in fp16
  - row-interleaved layout: plane k holds grid rows i with i % 4 == k at
    partition i // 4.  All four stencil neighbors are then FREE-dim
    shifts of one SBUF tile; the only cross-partition coupling is the
    plane-3 <-> plane-0 wrap, handled by two tiny "corner" matmuls.
  - per iteration: DVE computes the E/W pair sum; PE accumulates the
    raw neighbor sum into per-plane PSUM banks (up/down/corner/t
    matmuls, stationaries 1.0); delivery back to the fp16 state applies
    the 0.25 scale, split across ScalarE (activation-copy with scale,
    planes 0-1, b injected via 4I @ b4 matmuls) and DVE (STT
    (p*0.25)+b4, planes 2-3).  Per-plane PSUM double-buffering lets a
    plane's next accumulation start as soon as its delivery completes.
  - the iteration count is a RUNTIME input driving two hardware For_i
    loops (16-iteration body for the bulk + 2-iteration body for the
    remainder), so ONE NEFF (and one cached jit executable) serves
    every even maxiter; per-call wall time is flat in program size.
"""

import sys

sys.path.insert(0, "/opt/trn_rl_repo")

import numpy as np

_N = 512  # grid side
_PL = 4  # row planes per grid (row i -> plane i%4, partition i//4)
_P = 128  # partitions
_W = _N + 2  # padded row width (1 zero col each side)
_NCORES = 8
_BPC = 2  # batches per core


def _build_nc():
    import concourse.bacc as bacc
    import concourse.mybir as mybir
    from concourse.tile import TileContext
    from concourse.bass_types import OrderedSet

    f16 = mybir.dt.float16
    i32 = mybir.dt.int32
    nc = bacc.Bacc("TRN2", target_bir_lowering=False, debug=False, num_devices=_NCORES)

    u_in = nc.declare_dram_parameter("u", [_BPC, _P, _PL, _N], f16, isOutput=False)
    b_in = nc.declare_dram_parameter("b", [_BPC, _P, _PL, _N], f16, isOutput=False)
    im_in = nc.declare_dram_parameter("im", [_P, _P], f16, isOutput=False)
    cu_in = nc.declare_dram_parameter("cu", [_P, _P], f16, isOutput=False)
    cd_in = nc.declare_dram_parameter("cd", [_P, _P], f16, isOutput=False)
    im4_in = nc.declare_dram_parameter("im4", [_P, _P], f16, isOutput=False)
    mih_in = nc.declare_dram_parameter("mih", [1, 1], i32, isOutput=False)
    mil_in = nc.declare_dram_parameter("mil", [1, 1], i32, isOutput=False)
    out = nc.declare_dram_parameter("out", [_BPC, _P, _PL, _N], f16, isOutput=True)

    with TileContext(nc) as tc:
        with (
            tc.tile_pool(name="const", bufs=1) as const,
            tc.tile_pool(name="state", bufs=1) as state,
            tc.tile_pool(name="work", bufs=2) as work,
            tc.tile_pool(name="psum", bufs=2, space="PSUM") as psum,
        ):
            im = const.tile([_P, _P], f16, tag="im")
            cu = const.tile([_P, _P], f16, tag="cu")
            cd = const.tile([_P, _P], f16, tag="cd")
            im4 = const.tile([_P, _P], f16, tag="im4")
            nc.sync.dma_start(im[:], im_in[:])
            nc.sync.dma_start(cu[:], cu_in[:])
            nc.sync.dma_start(cd[:], cd_in[:])
            nc.sync.dma_start(im4[:], im4_in[:])

            x0s, x1s, bts = [], [], []
            for bi in range(_BPC):
                x0 = state.tile([_P, _PL, _W], f16, tag=f"x0_{bi}")
                x1 = state.tile([_P, _PL, _W], f16, tag=f"x1_{bi}")
                bt = state.tile([_P, _PL, _N], f16, tag=f"b{bi}")
                # zero so the pad columns stay zero forever (interior
                # rewrites never touch them)
                nc.gpsimd.memset(x0[:], 0.0)
                nc.gpsimd.memset(x1[:], 0.0)
                nc.sync.dma_start(x0[:, :, 1 : 1 + _N], u_in[bi])
                nc.sync.dma_start(bt[:], b_in[bi])
                x0s.append(x0)
                x1s.append(x1)
                bts.append(bt)

            def step(src, dst, bt):
                """One Jacobi update: dst = 0.25*(b + 4-neighbor sum of src).

                PSUM accumulates the RAW neighbor sum (stationaries 1.0);
                the 0.25 scale and the b term fold into the delivery ops so
                no per-step b matmuls are needed:
                  planes 0-1: b via PE matmuls, delivery = ACT
                              activation(Copy, scale=0.25)
                  planes 2-3: delivery = DVE STT (p * 0.25) + b4
                (bt holds b4 = 0.25*b precomputed on host; the 4I
                stationary im4 reconstructs raw b for planes 0-1.)
                """
                ps = []
                for g in range(_PL):
                    pg = psum.tile([_P, _N], mybir.dt.float32, tag=f"p{g}", name=f"pg{g}")
                    ps.append(pg)
                t = work.tile([_P, _PL, _N], f16, tag="t")
                # E/W pair sum on DVE (pad cols provide the zero boundary)
                nc.vector.tensor_add(t[:], src[:, :, 0:_N], src[:, :, 2 : 2 + _N])
                # a matmul's output must fit one PSUM bank (512 f32), so
                # each plane is its own accumulation group:
                # up (start) + down [+ b for plane 0] + t (stop)
                for g in range(_PL):
                    if g > 0:  # row i-1: plane g-1, same partition
                        nc.tensor.matmul(
                            ps[g][:], im[:], src[:, g - 1, 1 : 1 + _N],
                            start=True, stop=False,
                        )
                    else:  # wrap: plane 0 <- plane 3 shifted one partition
                        nc.tensor.matmul(
                            ps[0][:], cu[:], src[:, _PL - 1, 1 : 1 + _N],
                            start=True, stop=False,
                        )
                    if g < _PL - 1:  # row i+1: plane g+1, same partition
                        nc.tensor.matmul(
                            ps[g][:], im[:], src[:, g + 1, 1 : 1 + _N],
                            start=False, stop=False,
                        )
                    else:  # wrap: plane 3 <- plane 0 shifted one partition
                        nc.tensor.matmul(
                            ps[_PL - 1][:], cd[:], src[:, 0, 1 : 1 + _N],
                            start=False, stop=False,
                        )
                    if g < 2:
                        nc.tensor.matmul(
                            ps[g][:], im4[:], bt[:, g, :], start=False, stop=False
                        )
                    nc.tensor.matmul(
                        ps[g][:], im[:], t[:, g, :], start=False, stop=True
                    )
                # delivery: planes 0-1 on ACT (scaled copy), planes 2-3 on
                # DVE as (p * 0.25) + b4
                for g in range(2):
                    nc.scalar.activation(
                        dst[:, g, 1 : 1 + _N], ps[g][:],
                        mybir.ActivationFunctionType.Copy, 0.0, 0.25,
                    )
                for g in range(2, _PL):
                    nc.vector.scalar_tensor_tensor(
                        dst[:, g, 1 : 1 + _N], ps[g][:], 0.25,
                        bt[:, g, :],
                        mybir.AluOpType.mult, mybir.AluOpType.add,
                    )

            # two runtime-bounded loops: a 16-iteration body for the
            # bulk (amortizes the back-edge barrier and keeps the engine
            # pipelines full) and a 2-iteration body for the remainder,
            # so one NEFF serves any even maxiter = 16*mih + 2*mil.
            regs_h = nc.alloc_registers("mihr", OrderedSet(mybir.ALL_ENGINES))
            for reg in regs_h:
                nc.reg_load(reg, mih_in[0:1, 0:1])
            end_h = nc.snap(regs_h, min_val=0, max_val=1 << 20)
            regs_l = nc.alloc_registers("milr", OrderedSet(mybir.ALL_ENGINES))
            for reg in regs_l:
                nc.reg_load(reg, mil_in[0:1, 0:1])
            end_l = nc.snap(regs_l, min_val=0, max_val=1 << 20)

            with tc.For_i(0, end_h):
                for _ in range(8):
                    for bi in range(_BPC):
                        step(x0s[bi], x1s[bi], bts[bi])
                    for bi in range(_BPC):
                        step(x1s[bi], x0s[bi], bts[bi])
            with tc.For_i(0, end_l):
                for bi in range(_BPC):
                    step(x0s[bi], x1s[bi], bts[bi])
                for bi in range(_BPC):
                    step(x1s[bi], x0s[bi], bts[bi])

            for bi in range(_BPC):
                nc.sync.dma_start(out[bi], x0s[bi][:, :, 1 : 1 + _N])

    nc.finalize()
    return nc


_NC_CACHE: dict = {}


def _get_nc():
    if "nc" not in _NC_CACHE:
        _NC_CACHE["nc"] = _build_nc()
    return _NC_CACHE["nc"]


def _stencil_mats():
    # raw (1.0) stationaries: PSUM holds the unscaled neighbor sum and the
    # 0.25 scale is applied at delivery.  im4 = 4*I injects raw b from the
    # pre-scaled b4 = 0.25*b input.  matmul computes
    # out[p, c] = sum_q mat[q, p] * moving[q, c].
    idx = np.arange(_P - 1)
    im = np.eye(_P, dtype=np.float32)
    cu = np.zeros((_P, _P), np.float32)
    cu[idx, idx + 1] = 1.0  # out[p] += moving[p-1]  (plane0 <- plane3)
    cd = np.zeros((_P, _P), np.float32)
    cd[idx + 1, idx] = 1.0  # out[p] += moving[p+1]  (plane3 <- plane0)
    im4 = 4.0 * np.eye(_P, dtype=np.float32)
    return (
        im.astype(np.float16),
        cu.astype(np.float16),
        cd.astype(np.float16),
        im4.astype(np.float16),
    )


_EXPECTED_RC: list = []


def _verify_stencil(M_rows, M_cols, M_vals, invD):
    """Check the COO matrix is exactly the reference's 4-neighbor -1
    stencil (same entry order) and invD == 0.25 everywhere."""
    if not _EXPECTED_RC:
        g = np.arange(_N * _N, dtype=np.int32).reshape(_N, _N)
        rows = np.concatenate(
            [g[:, :-1].ravel(), g[:, 1:].ravel(), g[:-1, :].ravel(), g[1:, :].ravel()]
        )
        cols = np.concatenate(
            [g[:, 1:].ravel(), g[:, :-1].ravel(), g[1:, :].ravel(), g[:-1, :].ravel()]
        )
        _EXPECTED_RC.append((rows, cols))
    rows, cols = _EXPECTED_RC[0]
    r = np.asarray(M_rows)
    c = np.asarray(M_cols)
    v = np.asarray(M_vals)
    if r.shape != rows.shape or c.shape != cols.shape:
        return False
    return (
        np.array_equal(r, rows)
        and np.array_equal(c, cols)
        and np.all(v == np.float32(-1.0))
        and np.all(np.asarray(invD) == np.float32(0.25))
    )


def _fallback(u, b, M_rows, M_cols, M_vals, invD, maxiter):
    """Host path — only taken if inputs are not the expected stencil or
    maxiter is odd."""
    Bn = u.shape[0]
    n2 = int(np.asarray(u).size // Bn)
    rows = np.asarray(M_rows).astype(np.int64)
    cols = np.asarray(M_cols).astype(np.int64)
    vals = np.asarray(M_vals).astype(np.float32)
    x = np.asarray(u).reshape(Bn, -1).astype(np.float32)
    bb = np.asarray(b).astype(np.float32)
    iD = np.asarray(invD).astype(np.float32)
    try:
        from scipy.sparse import coo_matrix

        M = coo_matrix((vals, (rows, cols)), shape=(n2, n2)).tocsr()
        for _ in range(int(maxiter)):
            x = ((bb - (M @ x.T).T) * iD[None, :]).astype(np.float32)
    except ImportError:
        for _ in range(int(maxiter)):
            Mx = np.zeros_like(x)
            np.add.at(Mx, (slice(None), rows), vals[None, :] * x[:, cols])
            x = ((bb - Mx) * iD[None, :]).astype(np.float32)
    return x.reshape(u.shape)


_RUNNER_CACHE: dict = {}


def _get_runner(nc):
    """A cached multi-core PJRT runner (mirrors bass2jax.run_bass_via_pjrt
    but keeps the jitted executable alive across calls, so repeat calls
    pay only input transfer + execution)."""
    if "runner" in _RUNNER_CACHE:
        return _RUNNER_CACHE["runner"]

    import jax
    from jax.sharding import Mesh, PartitionSpec
    from jax.experimental.shard_map import shard_map
    from concourse import bass2jax, mybir

    bass2jax.install_neuronx_cc_hook()
    n_cores = _NCORES
    partition_name = nc.partition_id_tensor.name if nc.partition_id_tensor else None
    in_names, out_names, out_avals, zero_outs = [], [], [], []
    for alloc in nc.m.functions[0].allocations:
        if not isinstance(alloc, mybir.MemoryLocationSet):
            continue
        name = alloc.memorylocations[0].name
        if alloc.kind == "ExternalInput":
            if name != partition_name:
                in_names.append(name)
        elif alloc.kind == "ExternalOutput":
            shape = tuple(alloc.tensor_shape)
            dtype = mybir.dt.np(alloc.dtype)
            out_names.append(name)
            out_avals.append(jax.core.ShapedArray(shape, dtype))
            zero_outs.append(np.zeros((n_cores * shape[0], *shape[1:]), dtype))
    n_params = len(in_names)
    n_outs = len(out_avals)
    all_in_names = list(in_names) + list(out_names)
    if partition_name is not None:
        all_in_names.append(partition_name)

    def _body(*args):
        operands = list(args)
        if partition_name is not None:
            operands.append(bass2jax.partition_id_tensor())
        outs = bass2jax._bass_exec_p.bind(
            *operands,
            out_avals=tuple(out_avals),
            in_names=tuple(all_in_names),
            out_names=tuple(out_names),
            lowering_input_output_aliases=(),
            sim_require_finite=True,
            sim_require_nnan=True,
            nc=nc,
        )
        return tuple(outs)

    devices = jax.devices()[:n_cores]
    assert len(devices) == n_cores
    mesh = Mesh(np.asarray(devices), ("core",))
    donate = tuple(range(n_params, n_params + n_outs))
    in_specs = (PartitionSpec("core"),) * (n_params + n_outs)
    out_specs = (PartitionSpec("core"),) * n_outs
    sharded = jax.jit(
        shard_map(
            _body, mesh=mesh, in_specs=in_specs, out_specs=out_specs, check_rep=False
        ),
        donate_argnums=donate,
        keep_unused=True,
    )

    def run(in_maps):
        per_core = [[np.asarray(m[name]) for name in in_names] for m in in_maps]
        concat_in = [
            np.concatenate([per_core[c][i] for c in range(n_cores)], axis=0)
            for i in range(n_params)
        ]
        out_arrs = sharded(*concat_in, *zero_outs)
        return [
            {
                name: np.asarray(out_arrs[i]).reshape(
                    n_cores, *out_avals[i].shape
                )[c]
                for i, name in enumerate(out_names)
            }
            for c in range(n_cores)
        ]

    _RUNNER_CACHE["runner"] = run
    return run


def kernel(u, b, M_rows, M_cols, M_vals, invD, maxiter):
    u = np.asarray(u)
    b = np.asarray(b)
    mi = int(maxiter)

    Bn = u.shape[0]
    if (
        mi % 2 != 0
        or Bn != _NCORES * _BPC
        or not _verify_stencil(M_rows, M_cols, M_vals, invD)
    ):
        return _fallback(u, b, M_rows, M_cols, M_vals, invD, maxiter)

    nc = _get_nc()
    run = _get_runner(nc)
    im, cu, cd, im4 = _stencil_mats()
    # row-interleaved fp16 layout: [batch, p, k, j] with grid row = 4p+k
    u4 = np.ascontiguousarray(
        u.reshape(Bn, _P, _PL, _N).astype(np.float16)
    )
    b4 = np.ascontiguousarray(
        (0.25 * b.reshape(Bn, _P, _PL, _N)).astype(np.float16)
    )
    mih = np.full((1, 1), mi // 16, dtype=np.int32)
    mil = np.full((1, 1), (mi % 16) // 2, dtype=np.int32)

    in_maps = []
    for k in range(_NCORES):
        in_maps.append(
            {
                "u": u4[_BPC * k : _BPC * (k + 1)],
                "b": b4[_BPC * k : _BPC * (k + 1)],
                "im": im,
                "cu": cu,
                "cd": cd,
                "im4": im4,
                "mih": mih,
                "mil": mil,
            }
        )

    res = run(in_maps)
    outs = [res[k]["out"] for k in range(_NCORES)]
    full = np.concatenate(outs, axis=0)  # (Bn, P, PL, N) fp16
    return full.astype(np.float32).reshape(u.shape)


# revision 17
# speedup vs baseline: 1.0132x; 1.0132x over previous
"""Trainium2 Bass kernel for batched Jacobi iteration (5-point Laplacian).

Reference computation:
    x <- invD * (b - M x)   repeated `maxiter` times,
where M is the off-diagonal part of the 5-point Laplacian on a 512x512
grid, given in COO form.  For the actual inputs M is exactly the
4-neighbor stencil with value -1 and invD == 0.25, so the update is

    x_new[r, c] = 0.25 * (b[r, c] + x[r-1,c] + x[r+1,c] + x[r,c-1] + x[r,c+1])

(missing neighbors at grid edges contribute 0).

Strategy (8 NeuronCores, data parallel over batch B=16 -> 2 per core):
  - whole working set lives in SBUF for all iterations; state in fp16
  - row-interleaved layout: plane k holds grid rows i with i % 4 == k at
    partition i // 4.  All four stencil neighbors are then FREE-dim
    shifts of one SBUF tile; the only cross-partition coupling is the
    plane-3 <-> plane-0 wrap, handled by two tiny "corner" matmuls.
  - per iteration: DVE computes the E/W pair sum; PE accumulates the
    raw neighbor sum into per-plane PSUM banks (up/down/corner/t
    matmuls, stationaries 1.0); delivery back to the fp16 state applies
    the 0.25 scale, split across ScalarE (activation-copy with scale,
    planes 0-1, b injected via 4I @ b4 matmuls) and DVE (STT
    (p*0.25)+b4, planes 2-3).  Per-plane PSUM double-buffering lets a
    plane's next accumulation start as soon as its delivery completes.
  - the iteration count is a RUNTIME input driving two hardware For_i
    loops (16-iteration body for the bulk + 2-iteration body for the
    remainder), so ONE NEFF (and one cached jit executable) serves
    every even maxiter; per-call wall time is flat in program size.
"""

import sys

sys.path.insert(0, "/opt/trn_rl_repo")

import numpy as np

_N = 512  # grid side
_PL = 4  # row planes per grid (row i -> plane i%4, partition i//4)
_P = 128  # partitions
_W = _N + 2  # padded row width (1 zero col each side)
_NCORES = 8
_BPC = 2  # batches per core


def _build_nc():
    import concourse.bacc as bacc
    import concourse.mybir as mybir
    from concourse.tile import TileContext
    from concourse.bass_types import OrderedSet

    f16 = mybir.dt.float16
    i32 = mybir.dt.int32
    nc = bacc.Bacc("TRN2", target_bir_lowering=False, debug=False, num_devices=_NCORES)

    u_in = nc.declare_dram_parameter("u", [_BPC, _P, _PL, _N], f16, isOutput=False)
    b_in = nc.declare_dram_parameter("b", [_BPC, _P, _PL, _N], f16, isOutput=False)
    im_in = nc.declare_dram_parameter("im", [_P, _P], f16, isOutput=False)
    cu_in = nc.declare_dram_parameter("cu", [_P, _P], f16, isOutput=False)
    cd_in = nc.declare_dram_parameter("cd", [_P, _P], f16, isOutput=False)
    im4_in = nc.declare_dram_parameter("im4", [_P, _P], f16, isOutput=False)
    mih_in = nc.declare_dram_parameter("mih", [1, 1], i32, isOutput=False)
    mil_in = nc.declare_dram_parameter("mil", [1, 1], i32, isOutput=False)
    out = nc.declare_dram_parameter("out", [_BPC, _P, _PL, _N], f16, isOutput=True)

    with TileContext(nc) as tc:
        with (
            tc.tile_pool(name="const", bufs=1) as const,
            tc.tile_pool(name="state", bufs=1) as state,
            tc.tile_pool(name="work", bufs=4) as work,
            tc.tile_pool(name="psum", bufs=2, space="PSUM") as psum,
        ):
            im = const.tile([_P, _P], f16, tag="im")
            cu = const.tile([_P, _P], f16, tag="cu")
            cd = const.tile([_P, _P], f16, tag="cd")
            im4 = const.tile([_P, _P], f16, tag="im4")
            nc.sync.dma_start(im[:], im_in[:])
            nc.sync.dma_start(cu[:], cu_in[:])
            nc.sync.dma_start(cd[:], cd_in[:])
            nc.sync.dma_start(im4[:], im4_in[:])

            x0s, x1s, bts = [], [], []
            for bi in range(_BPC):
                x0 = state.tile([_P, _PL, _W], f16, tag=f"x0_{bi}")
                x1 = state.tile([_P, _PL, _W], f16, tag=f"x1_{bi}")
                bt = state.tile([_P, _PL, _N], f16, tag=f"b{bi}")
                # zero so the pad columns stay zero forever (interior
                # rewrites never touch them)
                nc.gpsimd.memset(x0[:], 0.0)
                nc.gpsimd.memset(x1[:], 0.0)
                nc.sync.dma_start(x0[:, :, 1 : 1 + _N], u_in[bi])
                nc.sync.dma_start(bt[:], b_in[bi])
                x0s.append(x0)
                x1s.append(x1)
                bts.append(bt)

            def step(src, dst, bt):
                """One Jacobi update: dst = 0.25*(b + 4-neighbor sum of src).

                PSUM accumulates the RAW neighbor sum (stationaries 1.0);
                the 0.25 scale and the b term fold into the delivery ops so
                no per-step b matmuls are needed:
                  planes 0-1: b via PE matmuls, delivery = ACT
                              activation(Copy, scale=0.25)
                  planes 2-3: delivery = DVE STT (p * 0.25) + b4
                (bt holds b4 = 0.25*b precomputed on host; the 4I
                stationary im4 reconstructs raw b for planes 0-1.)
                """
                ps = []
                for g in range(_PL):
                    pg = psum.tile([_P, _N], mybir.dt.float32, tag=f"p{g}", name=f"pg{g}")
                    ps.append(pg)
                t = work.tile([_P, _PL, _N], f16, tag="t")
                # E/W pair sum on DVE (pad cols provide the zero boundary)
                nc.vector.tensor_add(t[:], src[:, :, 0:_N], src[:, :, 2 : 2 + _N])
                # a matmul's output must fit one PSUM bank (512 f32), so
                # each plane is its own accumulation group:
                # up (start) + down [+ b for plane 0] + t (stop)
                for g in range(_PL):
                    if g > 0:  # row i-1: plane g-1, same partition
                        nc.tensor.matmul(
                            ps[g][:], im[:], src[:, g - 1, 1 : 1 + _N],
                            start=True, stop=False,
                        )
                    else:  # wrap: plane 0 <- plane 3 shifted one partition
                        nc.tensor.matmul(
                            ps[0][:], cu[:], src[:, _PL - 1, 1 : 1 + _N],
                            start=True, stop=False,
                        )
                    if g < _PL - 1:  # row i+1: plane g+1, same partition
                        nc.tensor.matmul(
                            ps[g][:], im[:], src[:, g + 1, 1 : 1 + _N],
                            start=False, stop=False,
                        )
                    else:  # wrap: plane 3 <- plane 0 shifted one partition
                        nc.tensor.matmul(
                            ps[_PL - 1][:], cd[:], src[:, 0, 1 : 1 + _N],
                            start=False, stop=False,
                        )
                    if g < 2:
                        nc.tensor.matmul(
                            ps[g][:], im4[:], bt[:, g, :], start=False, stop=False
                        )
                    nc.tensor.matmul(
                        ps[g][:], im[:], t[:, g, :], start=False, stop=True
                    )
                # delivery: planes 0-1 on ACT (scaled copy), planes 2-3 on
                # DVE as (p * 0.25) + b4
                for g in range(2):
                    nc.scalar.activation(
                        dst[:, g, 1 : 1 + _N], ps[g][:],
                        mybir.ActivationFunctionType.Copy, 0.0, 0.25,
                    )
                for g in range(2, _PL):
                    nc.vector.scalar_tensor_tensor(
                        dst[:, g, 1 : 1 + _N], ps[g][:], 0.25,
                        bt[:, g, :],
                        mybir.AluOpType.mult, mybir.AluOpType.add,
                    )

            # two runtime-bounded loops: a 16-iteration body for the
            # bulk (amortizes the back-edge barrier and keeps the engine
            # pipelines full) and a 2-iteration body for the remainder,
            # so one NEFF serves any even maxiter = 16*mih + 2*mil.
            regs_h = nc.alloc_registers("mihr", OrderedSet(mybir.ALL_ENGINES))
            for reg in regs_h:
                nc.reg_load(reg, mih_in[0:1, 0:1])
            end_h = nc.snap(regs_h, min_val=0, max_val=1 << 20)
            regs_l = nc.alloc_registers("milr", OrderedSet(mybir.ALL_ENGINES))
            for reg in regs_l:
                nc.reg_load(reg, mil_in[0:1, 0:1])
            end_l = nc.snap(regs_l, min_val=0, max_val=1 << 20)

            with tc.For_i(0, end_h):
                for _ in range(8):
                    for bi in range(_BPC):
                        step(x0s[bi], x1s[bi], bts[bi])
                    for bi in range(_BPC):
                        step(x1s[bi], x0s[bi], bts[bi])
            with tc.For_i(0, end_l):
                for bi in range(_BPC):
                    step(x0s[bi], x1s[bi], bts[bi])
                for bi in range(_BPC):
                    step(x1s[bi], x0s[bi], bts[bi])

            for bi in range(_BPC):
                nc.sync.dma_start(out[bi], x0s[bi][:, :, 1 : 1 + _N])

    nc.finalize()
    return nc


_NC_CACHE: dict = {}


def _get_nc():
    if "nc" not in _NC_CACHE:
        _NC_CACHE["nc"] = _build_nc()
    return _NC_CACHE["nc"]


def _stencil_mats():
    # raw (1.0) stationaries: PSUM holds the unscaled neighbor sum and the
    # 0.25 scale is applied at delivery.  im4 = 4*I injects raw b from the
    # pre-scaled b4 = 0.25*b input.  matmul computes
    # out[p, c] = sum_q mat[q, p] * moving[q, c].
    idx = np.arange(_P - 1)
    im = np.eye(_P, dtype=np.float32)
    cu = np.zeros((_P, _P), np.float32)
    cu[idx, idx + 1] = 1.0  # out[p] += moving[p-1]  (plane0 <- plane3)
    cd = np.zeros((_P, _P), np.float32)
    cd[idx + 1, idx] = 1.0  # out[p] += moving[p+1]  (plane3 <- plane0)
    im4 = 4.0 * np.eye(_P, dtype=np.float32)
    return (
        im.astype(np.float16),
        cu.astype(np.float16),
        cd.astype(np.float16),
        im4.astype(np.float16),
    )


_EXPECTED_RC: list = []


def _verify_stencil(M_rows, M_cols, M_vals, invD):
    """Check the COO matrix is exactly the reference's 4-neighbor -1
    stencil (same entry order) and invD == 0.25 everywhere."""
    if not _EXPECTED_RC:
        g = np.arange(_N * _N, dtype=np.int32).reshape(_N, _N)
        rows = np.concatenate(
            [g[:, :-1].ravel(), g[:, 1:].ravel(), g[:-1, :].ravel(), g[1:, :].ravel()]
        )
        cols = np.concatenate(
            [g[:, 1:].ravel(), g[:, :-1].ravel(), g[1:, :].ravel(), g[:-1, :].ravel()]
        )
        _EXPECTED_RC.append((rows, cols))
    rows, cols = _EXPECTED_RC[0]
    r = np.asarray(M_rows)
    c = np.asarray(M_cols)
    v = np.asarray(M_vals)
    if r.shape != rows.shape or c.shape != cols.shape:
        return False
    return (
        np.array_equal(r, rows)
        and np.array_equal(c, cols)
        and np.all(v == np.float32(-1.0))
        and np.all(np.asarray(invD) == np.float32(0.25))
    )


def _fallback(u, b, M_rows, M_cols, M_vals, invD, maxiter):
    """Host path — only taken if inputs are not the expected stencil or
    maxiter is odd."""
    Bn = u.shape[0]
    n2 = int(np.asarray(u).size // Bn)
    rows = np.asarray(M_rows).astype(np.int64)
    cols = np.asarray(M_cols).astype(np.int64)
    vals = np.asarray(M_vals).astype(np.float32)
    x = np.asarray(u).reshape(Bn, -1).astype(np.float32)
    bb = np.asarray(b).astype(np.float32)
    iD = np.asarray(invD).astype(np.float32)
    try:
        from scipy.sparse import coo_matrix

        M = coo_matrix((vals, (rows, cols)), shape=(n2, n2)).tocsr()
        for _ in range(int(maxiter)):
            x = ((bb - (M @ x.T).T) * iD[None, :]).astype(np.float32)
    except ImportError:
        for _ in range(int(maxiter)):
            Mx = np.zeros_like(x)
            np.add.at(Mx, (slice(None), rows), vals[None, :] * x[:, cols])
            x = ((bb - Mx) * iD[None, :]).astype(np.float32)
    return x.reshape(u.shape)


_RUNNER_CACHE: dict = {}


def _get_runner(nc):
    """A cached multi-core PJRT runner (mirrors bass2jax.run_bass_via_pjrt
    but keeps the jitted executable alive across calls, so repeat calls
    pay only input transfer + execution)."""
    if "runner" in _RUNNER_CACHE:
        return _RUNNER_CACHE["runner"]

    import jax
    from jax.sharding import Mesh, PartitionSpec
    from jax.experimental.shard_map import shard_map
    from concourse import bass2jax, mybir

    bass2jax.install_neuronx_cc_hook()
    n_cores = _NCORES
    partition_name = nc.partition_id_tensor.name if nc.partition_id_tensor else None
    in_names, out_names, out_avals, zero_outs = [], [], [], []
    for alloc in nc.m.functions[0].allocations:
        if not isinstance(alloc, mybir.MemoryLocationSet):
            continue
        name = alloc.memorylocations[0].name
        if alloc.kind == "ExternalInput":
            if name != partition_name:
                in_names.append(name)
        elif alloc.kind == "ExternalOutput":
            shape = tuple(alloc.tensor_shape)
            dtype = mybir.dt.np(alloc.dtype)
            out_names.append(name)
            out_avals.append(jax.core.ShapedArray(shape, dtype))
            zero_outs.append(np.zeros((n_cores * shape[0], *shape[1:]), dtype))
    n_params = len(in_names)
    n_outs = len(out_avals)
    all_in_names = list(in_names) + list(out_names)
    if partition_name is not None:
        all_in_names.append(partition_name)

    def _body(*args):
        operands = list(args)
        if partition_name is not None:
            operands.append(bass2jax.partition_id_tensor())
        outs = bass2jax._bass_exec_p.bind(
            *operands,
            out_avals=tuple(out_avals),
            in_names=tuple(all_in_names),
            out_names=tuple(out_names),
            lowering_input_output_aliases=(),
            sim_require_finite=True,
            sim_require_nnan=True,
            nc=nc,
        )
        return tuple(outs)

    devices = jax.devices()[:n_cores]
    assert len(devices) == n_cores
    mesh = Mesh(np.asarray(devices), ("core",))
    donate = tuple(range(n_params, n_params + n_outs))
    in_specs = (PartitionSpec("core"),) * (n_params + n_outs)
    out_specs = (PartitionSpec("core"),) * n_outs
    sharded = jax.jit(
        shard_map(
            _body, mesh=mesh, in_specs=in_specs, out_specs=out_specs, check_rep=False
        ),
        donate_argnums=donate,
        keep_unused=True,
    )

    def run(in_maps):
        per_core = [[np.asarray(m[name]) for name in in_names] for m in in_maps]
        concat_in = [
            np.concatenate([per_core[c][i] for c in range(n_cores)], axis=0)
            for i in range(n_params)
        ]
        out_arrs = sharded(*concat_in, *zero_outs)
        return [
            {
                name: np.asarray(out_arrs[i]).reshape(
                    n_cores, *out_avals[i].shape
                )[c]
                for i, name in enumerate(out_names)
            }
            for c in range(n_cores)
        ]

    _RUNNER_CACHE["runner"] = run
    return run


def kernel(u, b, M_rows, M_cols, M_vals, invD, maxiter):
    u = np.asarray(u)
    b = np.asarray(b)
    mi = int(maxiter)

    Bn = u.shape[0]
    if (
        mi % 2 != 0
        or Bn != _NCORES * _BPC
        or not _verify_stencil(M_rows, M_cols, M_vals, invD)
    ):
        return _fallback(u, b, M_rows, M_cols, M_vals, invD, maxiter)

    nc = _get_nc()
    run = _get_runner(nc)
    im, cu, cd, im4 = _stencil_mats()
    # row-interleaved fp16 layout: [batch, p, k, j] with grid row = 4p+k
    u4 = np.ascontiguousarray(
        u.reshape(Bn, _P, _PL, _N).astype(np.float16)
    )
    b4 = np.ascontiguousarray(
        (0.25 * b.reshape(Bn, _P, _PL, _N)).astype(np.float16)
    )
    mih = np.full((1, 1), mi // 16, dtype=np.int32)
    mil = np.full((1, 1), (mi % 16) // 2, dtype=np.int32)

    in_maps = []
    for k in range(_NCORES):
        in_maps.append(
            {
                "u": u4[_BPC * k : _BPC * (k + 1)],
                "b": b4[_BPC * k : _BPC * (k + 1)],
                "im": im,
                "cu": cu,
                "cd": cd,
                "im4": im4,
                "mih": mih,
                "mil": mil,
            }
        )

    res = run(in_maps)
    outs = [res[k]["out"] for k in range(_NCORES)]
    full = np.concatenate(outs, axis=0)  # (Bn, P, PL, N) fp16
    return full.astype(np.float32).reshape(u.shape)


# revision 18
# speedup vs baseline: 1.0464x; 1.0328x over previous
"""Trainium2 Bass kernel for batched Jacobi iteration (5-point Laplacian).

Reference computation:
    x <- invD * (b - M x)   repeated `maxiter` times,
where M is the off-diagonal part of the 5-point Laplacian on a 512x512
grid, given in COO form.  For the actual inputs M is exactly the
4-neighbor stencil with value -1 and invD == 0.25, so the update is

    x_new[r, c] = 0.25 * (b[r, c] + x[r-1,c] + x[r+1,c] + x[r,c-1] + x[r,c+1])

(missing neighbors at grid edges contribute 0).

Strategy (8 NeuronCores, data parallel over batch B=16 -> 2 per core):
  - whole working set lives in SBUF for all iterations; state in fp16
  - row-interleaved layout: plane k holds grid rows i with i % 4 == k at
    partition i // 4.  All four stencil neighbors are then FREE-dim
    shifts of one SBUF tile; the only cross-partition coupling is the
    plane-3 <-> plane-0 wrap, handled by two tiny "corner" matmuls.
  - per iteration: DVE computes the E/W pair sum; PE accumulates the
    raw neighbor sum into per-plane PSUM banks (up/down/corner/t
    matmuls, stationaries 1.0); delivery back to the fp16 state applies
    the 0.25 scale, split across ScalarE (activation-copy with scale,
    planes 0-1, b injected via 4I @ b4 matmuls) and DVE (STT
    (p*0.25)+b4, planes 2-3).  Per-plane PSUM double-buffering lets a
    plane's next accumulation start as soon as its delivery completes.
  - the iteration count is a RUNTIME input driving two hardware For_i
    loops (16-iteration body for the bulk + 2-iteration body for the
    remainder), so ONE NEFF (and one cached jit executable) serves
    every even maxiter; per-call wall time is flat in program size.
"""

import sys

sys.path.insert(0, "/opt/trn_rl_repo")

import numpy as np

_N = 512  # grid side
_PL = 4  # row planes per grid (row i -> plane i%4, partition i//4)
_P = 128  # partitions
_W = _N + 2  # padded row width (1 zero col each side)
_NCORES = 8
_BPC = 2  # batches per core


def _build_nc():
    import concourse.bacc as bacc
    import concourse.mybir as mybir
    from concourse.tile import TileContext
    from concourse.bass_types import OrderedSet

    f16 = mybir.dt.float16
    i32 = mybir.dt.int32
    nc = bacc.Bacc("TRN2", target_bir_lowering=False, debug=False, num_devices=_NCORES)

    u_in = nc.declare_dram_parameter("u", [_BPC, _P, _PL, _N], f16, isOutput=False)
    b_in = nc.declare_dram_parameter("b", [_BPC, _P, _PL, _N], f16, isOutput=False)
    im_in = nc.declare_dram_parameter("im", [_P, _P], f16, isOutput=False)
    cu_in = nc.declare_dram_parameter("cu", [_P, _P], f16, isOutput=False)
    cd_in = nc.declare_dram_parameter("cd", [_P, _P], f16, isOutput=False)
    im4_in = nc.declare_dram_parameter("im4", [_P, _P], f16, isOutput=False)
    mih_in = nc.declare_dram_parameter("mih", [1, 1], i32, isOutput=False)
    mil_in = nc.declare_dram_parameter("mil", [1, 1], i32, isOutput=False)
    out = nc.declare_dram_parameter("out", [_BPC, _P, _PL, _N], f16, isOutput=True)

    with TileContext(nc) as tc:
        with (
            tc.tile_pool(name="const", bufs=1) as const,
            tc.tile_pool(name="state", bufs=1) as state,
            tc.tile_pool(name="work", bufs=4) as work,
            tc.tile_pool(name="psum", bufs=2, space="PSUM") as psum,
        ):
            im = const.tile([_P, _P], f16, tag="im")
            cu = const.tile([_P, _P], f16, tag="cu")
            cd = const.tile([_P, _P], f16, tag="cd")
            im4 = const.tile([_P, _P], f16, tag="im4")
            nc.sync.dma_start(im[:], im_in[:])
            nc.sync.dma_start(cu[:], cu_in[:])
            nc.sync.dma_start(cd[:], cd_in[:])
            nc.sync.dma_start(im4[:], im4_in[:])

            x0s, x1s, bts = [], [], []
            for bi in range(_BPC):
                x0 = state.tile([_P, _PL, _W], f16, tag=f"x0_{bi}")
                x1 = state.tile([_P, _PL, _W], f16, tag=f"x1_{bi}")
                bt = state.tile([_P, _PL, _N], f16, tag=f"b{bi}")
                # zero so the pad columns stay zero forever (interior
                # rewrites never touch them)
                nc.gpsimd.memset(x0[:], 0.0)
                nc.gpsimd.memset(x1[:], 0.0)
                nc.sync.dma_start(x0[:, :, 1 : 1 + _N], u_in[bi])
                nc.sync.dma_start(bt[:], b_in[bi])
                x0s.append(x0)
                x1s.append(x1)
                bts.append(bt)

            def step(src, dst, bt):
                """One Jacobi update: dst = 0.25*(b + 4-neighbor sum of src).

                PSUM accumulates the RAW neighbor sum (stationaries 1.0);
                the 0.25 scale and the b term fold into the delivery ops so
                no per-step b matmuls are needed:
                  planes 0-1: b via PE matmuls, delivery = ACT
                              activation(Copy, scale=0.25)
                  planes 2-3: delivery = DVE STT (p * 0.25) + b4
                (bt holds b4 = 0.25*b precomputed on host; the 4I
                stationary im4 reconstructs raw b for planes 0-1.)
                """
                ps = []
                for g in range(_PL):
                    pg = psum.tile([_P, _N], mybir.dt.float32, tag=f"p{g}", name=f"pg{g}")
                    ps.append(pg)
                t = work.tile([_P, _PL, _N], f16, tag="t")
                # E/W pair sum on DVE (pad cols provide the zero boundary)
                nc.vector.tensor_add(t[:], src[:, :, 0:_N], src[:, :, 2 : 2 + _N])
                # a matmul's output must fit one PSUM bank (512 f32), so
                # each plane is its own accumulation group (start on its
                # first matmul, stop on its last).  Issue order groups
                # same-stationary matmuls to minimize stationary reloads:
                # all im up/down first, then cu/cd corners, im4 b-adds,
                # and the im t-injections (stop) last.
                for g in range(1, _PL):  # up, planes 1..3 (start)
                    nc.tensor.matmul(
                        ps[g][:], im[:], src[:, g - 1, 1 : 1 + _N],
                        start=True, stop=False,
                    )
                for g in range(_PL - 1):  # down, planes 0..2
                    nc.tensor.matmul(
                        ps[g][:], im[:], src[:, g + 1, 1 : 1 + _N],
                        start=(g == 0), stop=False,
                    )
                # wrap corners: plane 0 <- plane 3 (shifted down), plane 3
                # <- plane 0 (shifted up)
                nc.tensor.matmul(
                    ps[0][:], cu[:], src[:, _PL - 1, 1 : 1 + _N],
                    start=False, stop=False,
                )
                nc.tensor.matmul(
                    ps[_PL - 1][:], cd[:], src[:, 0, 1 : 1 + _N],
                    start=False, stop=False,
                )
                for g in range(2):  # raw b for the ACT-delivered planes
                    nc.tensor.matmul(
                        ps[g][:], im4[:], bt[:, g, :], start=False, stop=False
                    )
                for g in range(_PL):  # E/W sum injection (stop)
                    nc.tensor.matmul(
                        ps[g][:], im[:], t[:, g, :], start=False, stop=True
                    )
                # delivery: planes 0-1 on ACT (scaled copy), planes 2-3 on
                # DVE as (p * 0.25) + b4
                for g in range(2):
                    nc.scalar.activation(
                        dst[:, g, 1 : 1 + _N], ps[g][:],
                        mybir.ActivationFunctionType.Copy, 0.0, 0.25,
                    )
                for g in range(2, _PL):
                    nc.vector.scalar_tensor_tensor(
                        dst[:, g, 1 : 1 + _N], ps[g][:], 0.25,
                        bt[:, g, :],
                        mybir.AluOpType.mult, mybir.AluOpType.add,
                    )

            # two runtime-bounded loops: a 16-iteration body for the
            # bulk (amortizes the back-edge barrier and keeps the engine
            # pipelines full) and a 2-iteration body for the remainder,
            # so one NEFF serves any even maxiter = 16*mih + 2*mil.
            regs_h = nc.alloc_registers("mihr", OrderedSet(mybir.ALL_ENGINES))
            for reg in regs_h:
                nc.reg_load(reg, mih_in[0:1, 0:1])
            end_h = nc.snap(regs_h, min_val=0, max_val=1 << 20)
            regs_l = nc.alloc_registers("milr", OrderedSet(mybir.ALL_ENGINES))
            for reg in regs_l:
                nc.reg_load(reg, mil_in[0:1, 0:1])
            end_l = nc.snap(regs_l, min_val=0, max_val=1 << 20)

            with tc.For_i(0, end_h):
                for _ in range(8):
                    for bi in range(_BPC):
                        step(x0s[bi], x1s[bi], bts[bi])
                    for bi in range(_BPC):
                        step(x1s[bi], x0s[bi], bts[bi])
            with tc.For_i(0, end_l):
                for bi in range(_BPC):
                    step(x0s[bi], x1s[bi], bts[bi])
                for bi in range(_BPC):
                    step(x1s[bi], x0s[bi], bts[bi])

            for bi in range(_BPC):
                nc.sync.dma_start(out[bi], x0s[bi][:, :, 1 : 1 + _N])

    nc.finalize()
    return nc


_NC_CACHE: dict = {}


def _get_nc():
    if "nc" not in _NC_CACHE:
        _NC_CACHE["nc"] = _build_nc()
    return _NC_CACHE["nc"]


def _stencil_mats():
    # raw (1.0) stationaries: PSUM holds the unscaled neighbor sum and the
    # 0.25 scale is applied at delivery.  im4 = 4*I injects raw b from the
    # pre-scaled b4 = 0.25*b input.  matmul computes
    # out[p, c] = sum_q mat[q, p] * moving[q, c].
    idx = np.arange(_P - 1)
    im = np.eye(_P, dtype=np.float32)
    cu = np.zeros((_P, _P), np.float32)
    cu[idx, idx + 1] = 1.0  # out[p] += moving[p-1]  (plane0 <- plane3)
    cd = np.zeros((_P, _P), np.float32)
    cd[idx + 1, idx] = 1.0  # out[p] += moving[p+1]  (plane3 <- plane0)
    im4 = 4.0 * np.eye(_P, dtype=np.float32)
    return (
        im.astype(np.float16),
        cu.astype(np.float16),
        cd.astype(np.float16),
        im4.astype(np.float16),
    )


_EXPECTED_RC: list = []


def _verify_stencil(M_rows, M_cols, M_vals, invD):
    """Check the COO matrix is exactly the reference's 4-neighbor -1
    stencil (same entry order) and invD == 0.25 everywhere."""
    if not _EXPECTED_RC:
        g = np.arange(_N * _N, dtype=np.int32).reshape(_N, _N)
        rows = np.concatenate(
            [g[:, :-1].ravel(), g[:, 1:].ravel(), g[:-1, :].ravel(), g[1:, :].ravel()]
        )
        cols = np.concatenate(
            [g[:, 1:].ravel(), g[:, :-1].ravel(), g[1:, :].ravel(), g[:-1, :].ravel()]
        )
        _EXPECTED_RC.append((rows, cols))
    rows, cols = _EXPECTED_RC[0]
    r = np.asarray(M_rows)
    c = np.asarray(M_cols)
    v = np.asarray(M_vals)
    if r.shape != rows.shape or c.shape != cols.shape:
        return False
    return (
        np.array_equal(r, rows)
        and np.array_equal(c, cols)
        and np.all(v == np.float32(-1.0))
        and np.all(np.asarray(invD) == np.float32(0.25))
    )


def _fallback(u, b, M_rows, M_cols, M_vals, invD, maxiter):
    """Host path — only taken if inputs are not the expected stencil or
    maxiter is odd."""
    Bn = u.shape[0]
    n2 = int(np.asarray(u).size // Bn)
    rows = np.asarray(M_rows).astype(np.int64)
    cols = np.asarray(M_cols).astype(np.int64)
    vals = np.asarray(M_vals).astype(np.float32)
    x = np.asarray(u).reshape(Bn, -1).astype(np.float32)
    bb = np.asarray(b).astype(np.float32)
    iD = np.asarray(invD).astype(np.float32)
    try:
        from scipy.sparse import coo_matrix

        M = coo_matrix((vals, (rows, cols)), shape=(n2, n2)).tocsr()
        for _ in range(int(maxiter)):
            x = ((bb - (M @ x.T).T) * iD[None, :]).astype(np.float32)
    except ImportError:
        for _ in range(int(maxiter)):
            Mx = np.zeros_like(x)
            np.add.at(Mx, (slice(None), rows), vals[None, :] * x[:, cols])
            x = ((bb - Mx) * iD[None, :]).astype(np.float32)
    return x.reshape(u.shape)


_RUNNER_CACHE: dict = {}


def _get_runner(nc):
    """A cached multi-core PJRT runner (mirrors bass2jax.run_bass_via_pjrt
    but keeps the jitted executable alive across calls, so repeat calls
    pay only input transfer + execution)."""
    if "runner" in _RUNNER_CACHE:
        return _RUNNER_CACHE["runner"]

    import jax
    from jax.sharding import Mesh, PartitionSpec
    from jax.experimental.shard_map import shard_map
    from concourse import bass2jax, mybir

    bass2jax.install_neuronx_cc_hook()
    n_cores = _NCORES
    partition_name = nc.partition_id_tensor.name if nc.partition_id_tensor else None
    in_names, out_names, out_avals, zero_outs = [], [], [], []
    for alloc in nc.m.functions[0].allocations:
        if not isinstance(alloc, mybir.MemoryLocationSet):
            continue
        name = alloc.memorylocations[0].name
        if alloc.kind == "ExternalInput":
            if name != partition_name:
                in_names.append(name)
        elif alloc.kind == "ExternalOutput":
            shape = tuple(alloc.tensor_shape)
            dtype = mybir.dt.np(alloc.dtype)
            out_names.append(name)
            out_avals.append(jax.core.ShapedArray(shape, dtype))
            zero_outs.append(np.zeros((n_cores * shape[0], *shape[1:]), dtype))
    n_params = len(in_names)
    n_outs = len(out_avals)
    all_in_names = list(in_names) + list(out_names)
    if partition_name is not None:
        all_in_names.append(partition_name)

    def _body(*args):
        operands = list(args)
        if partition_name is not None:
            operands.append(bass2jax.partition_id_tensor())
        outs = bass2jax._bass_exec_p.bind(
            *operands,
            out_avals=tuple(out_avals),
            in_names=tuple(all_in_names),
            out_names=tuple(out_names),
            lowering_input_output_aliases=(),
            sim_require_finite=True,
            sim_require_nnan=True,
            nc=nc,
        )
        return tuple(outs)

    devices = jax.devices()[:n_cores]
    assert len(devices) == n_cores
    mesh = Mesh(np.asarray(devices), ("core",))
    donate = tuple(range(n_params, n_params + n_outs))
    in_specs = (PartitionSpec("core"),) * (n_params + n_outs)
    out_specs = (PartitionSpec("core"),) * n_outs
    sharded = jax.jit(
        shard_map(
            _body, mesh=mesh, in_specs=in_specs, out_specs=out_specs, check_rep=False
        ),
        donate_argnums=donate,
        keep_unused=True,
    )

    def run(in_maps):
        per_core = [[np.asarray(m[name]) for name in in_names] for m in in_maps]
        concat_in = [
            np.concatenate([per_core[c][i] for c in range(n_cores)], axis=0)
            for i in range(n_params)
        ]
        out_arrs = sharded(*concat_in, *zero_outs)
        return [
            {
                name: np.asarray(out_arrs[i]).reshape(
                    n_cores, *out_avals[i].shape
                )[c]
                for i, name in enumerate(out_names)
            }
            for c in range(n_cores)
        ]

    _RUNNER_CACHE["runner"] = run
    return run


def kernel(u, b, M_rows, M_cols, M_vals, invD, maxiter):
    u = np.asarray(u)
    b = np.asarray(b)
    mi = int(maxiter)

    Bn = u.shape[0]
    if (
        mi % 2 != 0
        or Bn != _NCORES * _BPC
        or not _verify_stencil(M_rows, M_cols, M_vals, invD)
    ):
        return _fallback(u, b, M_rows, M_cols, M_vals, invD, maxiter)

    nc = _get_nc()
    run = _get_runner(nc)
    im, cu, cd, im4 = _stencil_mats()
    # row-interleaved fp16 layout: [batch, p, k, j] with grid row = 4p+k
    u4 = np.ascontiguousarray(
        u.reshape(Bn, _P, _PL, _N).astype(np.float16)
    )
    b4 = np.ascontiguousarray(
        (0.25 * b.reshape(Bn, _P, _PL, _N)).astype(np.float16)
    )
    mih = np.full((1, 1), mi // 16, dtype=np.int32)
    mil = np.full((1, 1), (mi % 16) // 2, dtype=np.int32)

    in_maps = []
    for k in range(_NCORES):
        in_maps.append(
            {
                "u": u4[_BPC * k : _BPC * (k + 1)],
                "b": b4[_BPC * k : _BPC * (k + 1)],
                "im": im,
                "cu": cu,
                "cd": cd,
                "im4": im4,
                "mih": mih,
                "mil": mil,
            }
        )

    res = run(in_maps)
    outs = [res[k]["out"] for k in range(_NCORES)]
    full = np.concatenate(outs, axis=0)  # (Bn, P, PL, N) fp16
    return full.astype(np.float32).reshape(u.shape)
